# revision 8
# baseline (speedup 1.0000x reference)
"""Trainium2 Bass kernel for pointer-generator attention.

Problem (B=32, TK=2048, N=1024):
    enc_feat = h @ W_h.T                [B,TK,N]
    q_feat   = q_h @ W_q.T              [B,TK,N]
    dec_feat = s_t_hat @ W_d.T + b_d    [B,N]
    att  = tanh(enc_feat + dec_feat[:,None,:] + q_feat + coverage[...,None]*W_c)
    scores = att @ v_w                  [B,TK]
    attn = softmax(scores)*mask renormalized
    c_t  = attn @ h                     [B,N]
    coverage_out = coverage + attn

Strategy: data-parallel over batch, 4 batches per core on 8 cores.
Device kernel per batch (all matmuls in fp32r = bf16-rate with ~fp32
accuracy; weights stationary, host-pretransposed hT/qT as moving ops):
  - per 512-token tile: z^T[m,t] accumulated in PSUM over 16 K=128
    chunks (h and q) + a K=1 rank-1 matmul for coverage*W_c;
    dec_feat folded in as the per-partition bias of the tanh ACT op;
    scores via PE matvec with v stationary;
    wm = exp(scores)*mask; c_t partial-accumulated with UNNORMALIZED wm
    (renormalization folded into the final copy as a 1/sum scale).
"""
import numpy as np

B, TK, N = 32, 2048, 1024
NCORES = 8
BPC = B // NCORES       # batches per core
TT = 512                # token tile
NTT = TK // TT          # 4 token tiles per batch
NCH = N // 128          # 8 contraction chunks
MCH = N // 128          # 8 output chunks

_cache = {}


def _build():
    from contextlib import ExitStack
    import concourse.tile as tile
    from concourse import bacc, mybir
    from concourse.tile_rust import add_dep_helper

    F32 = mybir.dt.float32
    F32R = mybir.dt.float32r
    AF = mybir.ActivationFunctionType
    ALU = mybir.AluOpType

    nc = bacc.Bacc("TRN2", target_bir_lowering=False, debug=False,
                   enable_asserts=False, num_devices=NCORES)

    # ---- DRAM I/O ----
    hT_d = nc.dram_tensor("hT", [BPC, N, TK], F32R, kind="ExternalInput").ap()
    qT_d = nc.dram_tensor("qT", [BPC, N, TK], F32R, kind="ExternalInput").ap()
    h_d = nc.dram_tensor("h", [BPC, TK, N], F32R, kind="ExternalInput").ap()
    WhT_d = nc.dram_tensor("WhT", [N, N], F32R, kind="ExternalInput").ap()
    WqT_d = nc.dram_tensor("WqT", [N, N], F32R, kind="ExternalInput").ap()
    cov_d = nc.dram_tensor("cov", [1, BPC * TK], F32R, kind="ExternalInput").ap()
    mask_d = nc.dram_tensor("mask", [1, BPC * TK], F32, kind="ExternalInput").ap()
    dec_d = nc.dram_tensor("dec", [128, BPC * MCH], F32, kind="ExternalInput").ap()
    v_d = nc.dram_tensor("v", [128, NCH], F32R, kind="ExternalInput").ap()
    Wc_d = nc.dram_tensor("Wc", [1, N], F32R, kind="ExternalInput").ap()

    ct_o = nc.dram_tensor("ct", [BPC, N], F32, kind="ExternalOutput").ap()
    attn_o = nc.dram_tensor("attn", [BPC, TK], F32, kind="ExternalOutput").ap()
    cvo_o = nc.dram_tensor("cvo", [BPC, TK], F32, kind="ExternalOutput").ap()

    with tile.TileContext(nc) as tc, ExitStack() as ctx:
        wp = ctx.enter_context(tc.tile_pool(name="wp", bufs=1))
        hTp = ctx.enter_context(tc.tile_pool(name="hTp", bufs=2))
        qTp = ctx.enter_context(tc.tile_pool(name="qTp", bufs=2))
        hp = ctx.enter_context(tc.tile_pool(name="hp", bufs=2))
        thp = ctx.enter_context(tc.tile_pool(name="thp", bufs=3))
        smp = ctx.enter_context(tc.tile_pool(name="smp", bufs=2))
        ps_at = ctx.enter_context(tc.tile_pool(name="ps_at", bufs=2, space="PSUM"))
        ps_sc = ctx.enter_context(tc.tile_pool(name="ps_sc", bufs=2, space="PSUM"))
        ps_tp = ctx.enter_context(tc.tile_pool(name="ps_tp", bufs=2, space="PSUM"))
        ps_ct = ctx.enter_context(tc.tile_pool(name="ps_ct", bufs=2, space="PSUM"))

        # ---- persistent weights / small tensors ----
        WhT = wp.tile([128, NCH, N], F32R)
        WqT = wp.tile([128, NCH, N], F32R)
        WhT_r = WhT_d.rearrange("(j p) m -> p j m", p=128)
        WqT_r = WqT_d.rearrange("(j p) m -> p j m", p=128)
        v_sb = wp.tile([128, NCH], F32R)
        Wc_sb = wp.tile([1, N], F32R)
        dec_sb = wp.tile([128, BPC * MCH], F32)
        ident1 = wp.tile([1, 1], F32)
        # startup ordering: W arrives in m-slices so group m can start as
        # soon as slice m + the first token tiles are in (~14us), instead
        # of waiting for the full 8MB of weights
        nc.sync.dma_start(out=WhT[:, :, 0:128], in_=WhT_r[:, :, 0:128])
        nc.sync.dma_start(out=WqT[:, :, 0:128], in_=WqT_r[:, :, 0:128])
        nc.sync.dma_start(out=v_sb, in_=v_d)
        nc.sync.dma_start(out=Wc_sb, in_=Wc_d)
        nc.sync.dma_start(out=dec_sb, in_=dec_d)
        nc.vector.memset(ident1, 1.0)
        first_tiles = {}
        hT_t0 = hTp.tile([128, NCH, TT], F32R, tag="hT_t")
        nc.sync.dma_start(out=hT_t0, in_=hT_d[0].rearrange("(j p) t -> p j t", p=128)[:, :, 0:TT])
        qT_t0 = qTp.tile([128, NCH, TT], F32R, tag="qT_t")
        nc.sync.dma_start(out=qT_t0, in_=qT_d[0].rearrange("(j p) t -> p j t", p=128)[:, :, 0:TT])
        first_tiles[0] = (hT_t0, qT_t0)
        for m in range(1, MCH):
            msl = slice(m * 128, (m + 1) * 128)
            nc.sync.dma_start(out=WhT[:, :, msl], in_=WhT_r[:, :, msl])
            nc.sync.dma_start(out=WqT[:, :, msl], in_=WqT_r[:, :, msl])

        for b in range(BPC):
            wma = smp.tile([1, TK], F32, tag="wma")
            cov_t = smp.tile([1, TK], F32R, tag="cov")
            nc.sync.dma_start(out=cov_t, in_=cov_d[0:1, b * TK:(b + 1) * TK])
            ct0 = ps_ct.tile([1, 512], F32, tag="ct")
            ct1 = ps_ct.tile([1, 512], F32, tag="ct")

            # deferred-emission helper: c_t partial for tile tt (runs one
            # tt later so the exp/mask/transpose chain is off PE's critical
            # path)
            def emit_ct_partial(b, tt, wma=wma, ct0=ct0, ct1=ct1):
                sl = slice(tt * TT, (tt + 1) * TT)
                mk = thp.tile([1, TT], F32, tag="mk")
                nc.sync.dma_start(
                    out=mk, in_=mask_d[0:1, b * TK + tt * TT: b * TK + (tt + 1) * TT])
                # wm = exp(scores) * mask  (in place)
                nc.vector.tensor_tensor(
                    out=wma[0:1, sl], in0=wma[0:1, sl], in1=mk, op=ALU.mult)
                # transpose wm tile -> [128, 4] chunks
                tp_ps = ps_tp.tile([128, 4], F32, tag="tp")
                for i in range(4):
                    nc.tensor.transpose(
                        tp_ps[:, i:i + 1],
                        wma[0:1, tt * TT + i * 128: tt * TT + (i + 1) * 128],
                        ident1)
                wmT = thp.tile([128, 4], F32R, tag="wmT")
                nc.scalar.activation(wmT, tp_ps, AF.Identity)
                # c_t partial accumulation
                for half in range(2):
                    h_t = hp.tile([128, 2, N], F32R, tag="h_t")
                    nc.sync.dma_start(
                        out=h_t,
                        in_=h_d[b].rearrange("(u p) n -> p u n", p=128)[
                            :, tt * 4 + half * 2: tt * 4 + half * 2 + 2, :])
                    for i in range(2):
                        u = half * 2 + i
                        nc.tensor.matmul(ct0, wmT[:, u:u + 1], h_t[:, i, 0:512],
                                         start=(tt == 0 and u == 0),
                                         stop=(tt == NTT - 1 and u == 3))
                        nc.tensor.matmul(ct1, wmT[:, u:u + 1], h_t[:, i, 512:1024],
                                         start=(tt == 0 and u == 0),
                                         stop=(tt == NTT - 1 and u == 3))

            # pending scores matvec, emitted one m-group behind so the
            # tanh (ACT) never stalls the PE: (sc_ps, m, tt, th_t)
            pending = []

            def flush_pending(after=None, pending=pending, wma=wma):
                sc_ps, m, tt, th_t = pending.pop()
                bi = nc.tensor.matmul(sc_ps, v_sb[:, m:m + 1], th_t[:],
                                      start=(m == 0), stop=(m == MCH - 1))
                if after is not None:
                    add_dep_helper(bi.ins, after.ins, sync=False,
                                   reason="hold scores matvec behind next group")
                if m == MCH - 1:
                    nc.scalar.activation(
                        wma[0:1, tt * TT:(tt + 1) * TT], sc_ps, AF.Exp)

            sc_ps = None
            for tt in range(NTT):
                sl = slice(tt * TT, (tt + 1) * TT)
                if b == 0 and tt == 0:
                    hT_t, qT_t = first_tiles.pop(0)
                else:
                    hT_t = hTp.tile([128, NCH, TT], F32R, tag="hT_t")
                    nc.sync.dma_start(
                        out=hT_t,
                        in_=hT_d[b].rearrange("(j p) t -> p j t", p=128)[:, :, sl])
                    qT_t = qTp.tile([128, NCH, TT], F32R, tag="qT_t")
                    nc.sync.dma_start(
                        out=qT_t,
                        in_=qT_d[b].rearrange("(j p) t -> p j t", p=128)[:, :, sl])

                sc_ps = ps_sc.tile([1, TT], F32, tag="sc")
                for m in range(MCH):
                    at_ps = ps_at.tile([128, TT], F32, tag="at")
                    msl = slice(m * 128, (m + 1) * 128)
                    nc.tensor.matmul(at_ps, WhT[:, 0, msl], hT_t[:, 0, :],
                                     start=True, stop=False)
                    nc.tensor.matmul(at_ps, Wc_sb[0:1, msl], cov_t[0:1, sl],
                                     start=False, stop=False)
                    mm8 = None
                    for j in range(1, NCH):
                        bi = nc.tensor.matmul(at_ps, WhT[:, j, msl], hT_t[:, j, :],
                                              start=False, stop=False)
                        if j == NCH - 1:
                            mm8 = bi
                    for j in range(NCH):
                        nc.tensor.matmul(at_ps, WqT[:, j, msl], qT_t[:, j, :],
                                         start=False, stop=(j == NCH - 1))
                    if pending:
                        flush_pending(after=mm8)
                    th_t = thp.tile([128, TT], F32R, tag="th")
                    nc.scalar.activation(
                        th_t, at_ps, AF.Tanh,
                        bias=dec_sb[:, b * MCH + m: b * MCH + m + 1])
                    pending.append((sc_ps, m, tt, th_t))
                    # slot the deferred c_t work of the previous token tile
                    # behind this tile's projections
                    if m == 2 and tt > 0:
                        emit_ct_partial(b, tt - 1)

            flush_pending()
            emit_ct_partial(b, NTT - 1)

            # ---- softmax tail for batch b ----
            tot = smp.tile([1, 1], F32, tag="tot")
            nc.vector.reduce_sum(out=tot, in_=wma, axis=mybir.AxisListType.X)
            rt = smp.tile([1, 1], F32, tag="rt")
            nc.vector.reciprocal(rt, tot)
            nc.vector.tensor_scalar_mul(wma, wma, rt[0:1, 0:1])
            nc.sync.dma_start(out=attn_o[b], in_=wma)
            nc.vector.tensor_add(wma, wma, cov_t[:].bitcast(F32))
            nc.sync.dma_start(out=cvo_o[b], in_=wma)
            cts = smp.tile([1, N], F32, tag="cts")
            nc.scalar.activation(cts[0:1, 0:512], ct0, AF.Copy, scale=rt[0:1, 0:1])
            nc.scalar.activation(cts[0:1, 512:1024], ct1, AF.Copy, scale=rt[0:1, 0:1])
            nc.sync.dma_start(out=ct_o[b], in_=cts)

    nc.compile()
    return nc


def kernel(s_t_hat, h, enc_padding_mask, coverage, q_h, W_h, W_q, W_c, W_d, b_d, v_w):
    import os
    import jax
    from concourse import bass_utils

    try:
        jax.config.update("jax_compilation_cache_dir", "/tmp/jax_kernel_cache")
        jax.config.update("jax_persistent_cache_min_compile_time_secs", 0.0)
    except Exception:
        pass

    if "nc" not in _cache:
        _cache["nc"] = _build()
    nc = _cache["nc"]

    f32 = np.float32
    h = np.asarray(h, f32)
    q_h = np.asarray(q_h, f32)
    coverage = np.asarray(coverage, f32)
    enc_padding_mask = np.asarray(enc_padding_mask, f32)

    # host-side prep
    hT = np.ascontiguousarray(h.transpose(0, 2, 1))          # [B, N, TK]
    qT = np.ascontiguousarray(q_h.transpose(0, 2, 1))
    dec = (np.asarray(s_t_hat, f32) @ np.asarray(W_d, f32).T
           + np.asarray(b_d, f32))                           # [B, N]
    WhT = np.ascontiguousarray(np.asarray(W_h, f32).T)       # [n, m]
    WqT = np.ascontiguousarray(np.asarray(W_q, f32).T)
    v_arr = np.ascontiguousarray(np.asarray(v_w, f32).reshape(NCH, 128).T)
    Wc_arr = np.ascontiguousarray(np.asarray(W_c, f32).reshape(1, N))

    in_maps = []
    for c in range(NCORES):
        bs = slice(c * BPC, (c + 1) * BPC)
        dec_c = np.ascontiguousarray(
            dec[bs].reshape(BPC, MCH, 128).transpose(2, 0, 1).reshape(128, BPC * MCH))
        in_maps.append(dict(
            hT=np.ascontiguousarray(hT[bs]),
            qT=np.ascontiguousarray(qT[bs]),
            h=np.ascontiguousarray(h[bs]),
            WhT=WhT, WqT=WqT,
            cov=np.ascontiguousarray(coverage[bs].reshape(1, BPC * TK)),
            mask=np.ascontiguousarray(enc_padding_mask[bs].reshape(1, BPC * TK)),
            dec=dec_c, v=v_arr, Wc=Wc_arr,
        ))

    _cache["in_maps"] = in_maps
    res = bass_utils.run_bass_kernel_spmd(
        nc, in_maps, core_ids=list(range(NCORES)),
        trace=bool(os.environ.get("KERNEL_TRACE")))
    _cache["last_result"] = res

    c_t = np.concatenate([r["ct"] for r in res.results], axis=0)
    attn = np.concatenate([r["attn"] for r in res.results], axis=0)
    cvo = np.concatenate([r["cvo"] for r in res.results], axis=0)
    return (c_t, attn, cvo)


# revision 18
# speedup vs baseline: 1.0967x; 1.0967x over previous
"""Trainium2 Bass kernel for pointer-generator attention.

Problem (B=32, TK=2048, N=1024):
    enc_feat = h @ W_h.T                [B,TK,N]
    q_feat   = q_h @ W_q.T              [B,TK,N]
    dec_feat = s_t_hat @ W_d.T + b_d    [B,N]
    att  = tanh(enc_feat + dec_feat[:,None,:] + q_feat + coverage[...,None]*W_c)
    scores = att @ v_w                  [B,TK]
    attn = softmax(scores)*mask renormalized
    c_t  = attn @ h                     [B,N]
    coverage_out = coverage + attn

Strategy: data-parallel over batch, 4 batches per core on 8 cores.
Device kernel per batch (all matmuls in fp32r = bf16-rate with ~fp32
accuracy; weights stationary, host-pretransposed hT/qT as moving ops):
  - per 512-token tile: z^T[m,t] accumulated in PSUM over 16 K=128
    chunks (h and q) + a zero-padded K=128 rank-1 matmul for
    coverage*W_c (K=1 stationaries break LDWEIGHTS double-buffering);
    dec_feat folded in as the per-partition bias of the tanh ACT op;
    scores via PE matvec with v stationary;
    wm = exp(scores)*mask; c_t partial-accumulated with UNNORMALIZED wm
    (renormalization folded into the final copy as a 1/sum scale).
"""
import numpy as np

B, TK, N = 32, 2048, 1024
NCORES = 8
BPC = B // NCORES       # batches per core
TT = 512                # token tile
NTT = TK // TT          # 4 token tiles per batch
NCH = N // 128          # 8 contraction chunks
MCH = N // 128          # 8 output chunks

_cache = {}


def _build():
    from contextlib import ExitStack
    import concourse.tile as tile
    from concourse import bacc, mybir
    from concourse.tile_rust import add_dep_helper

    F32 = mybir.dt.float32
    F32R = mybir.dt.float32r
    AF = mybir.ActivationFunctionType
    ALU = mybir.AluOpType

    nc = bacc.Bacc("TRN2", target_bir_lowering=False, debug=False,
                   enable_asserts=False, num_devices=NCORES)
    mmtags = _cache.setdefault("mmtags", {})

    def tg(bi, s):
        mmtags[bi.ins.name] = s
        return bi

    # ---- DRAM I/O ----
    hT_d = nc.dram_tensor("hT", [BPC, NTT, NCH, 128, TT], F32R, kind="ExternalInput").ap()
    qT_d = nc.dram_tensor("qT", [BPC, NTT, NCH, 128, TT], F32R, kind="ExternalInput").ap()
    h_d = nc.dram_tensor("h", [BPC, TK, N], F32R, kind="ExternalInput").ap()
    WhT_d = nc.dram_tensor("WhT", [MCH, NCH, 128, 128], F32R, kind="ExternalInput").ap()
    WqT_d = nc.dram_tensor("WqT", [MCH, NCH, 128, 128], F32R, kind="ExternalInput").ap()
    cov_d = nc.dram_tensor("cov", [1, BPC * TK], F32R, kind="ExternalInput").ap()
    mask_d = nc.dram_tensor("mask", [1, BPC * TK], F32, kind="ExternalInput").ap()
    dec_d = nc.dram_tensor("dec", [128, BPC * MCH], F32, kind="ExternalInput").ap()
    v_d = nc.dram_tensor("v", [128, NCH], F32R, kind="ExternalInput").ap()
    Wc_d = nc.dram_tensor("Wc", [1, N], F32R, kind="ExternalInput").ap()

    ct_o = nc.dram_tensor("ct", [BPC, N], F32, kind="ExternalOutput").ap()
    attn_o = nc.dram_tensor("attn", [BPC, TK], F32, kind="ExternalOutput").ap()
    cvo_o = nc.dram_tensor("cvo", [BPC, TK], F32, kind="ExternalOutput").ap()

    with tile.TileContext(nc) as tc, ExitStack() as ctx:
        wp = ctx.enter_context(tc.tile_pool(name="wp", bufs=1))
        hTp = ctx.enter_context(tc.tile_pool(name="hTp", bufs=2))
        qTp = ctx.enter_context(tc.tile_pool(name="qTp", bufs=2))
        hp = ctx.enter_context(tc.tile_pool(name="hp", bufs=2))
        thp = ctx.enter_context(tc.tile_pool(name="thp", bufs=3))
        smp = ctx.enter_context(tc.tile_pool(name="smp", bufs=2))
        ps_at = ctx.enter_context(tc.tile_pool(name="ps_at", bufs=2, space="PSUM"))
        ps_sc = ctx.enter_context(tc.tile_pool(name="ps_sc", bufs=2, space="PSUM"))
        ps_tp = ctx.enter_context(tc.tile_pool(name="ps_tp", bufs=2, space="PSUM"))
        ps_ct = ctx.enter_context(tc.tile_pool(name="ps_ct", bufs=2, space="PSUM"))

        # ---- persistent weights / small tensors ----
        WhT = wp.tile([128, NCH, N], F32R)
        WqT = wp.tile([128, NCH, N], F32R)
        v_sb = wp.tile([128, NCH], F32R)
        Wc_sb = wp.tile([128, N], F32R)
        nc.vector.memset(Wc_sb[:].bitcast(F32), 0.0)
        dec_sb = wp.tile([128, BPC * MCH], F32)
        ident1 = wp.tile([1, 1], F32)
        # startup ordering: W arrives in m-slices so group m can start as
        # soon as slice m + the first token tiles are in (~14us), instead
        # of waiting for the full 8MB of weights
        WhT_v = WhT.rearrange("p j (m k) -> p j m k", k=128)
        WqT_v = WqT.rearrange("p j (m k) -> p j m k", k=128)
        nc.sync.dma_start(out=WhT_v[:, :, 0, :], in_=WhT_d[0].rearrange("j p k -> p j k"))
        nc.scalar.dma_start(out=WqT_v[:, :, 0, :], in_=WqT_d[0].rearrange("j p k -> p j k"))
        nc.gpsimd.dma_start(out=v_sb, in_=v_d)
        nc.gpsimd.dma_start(out=Wc_sb[0:1, :], in_=Wc_d)
        nc.gpsimd.dma_start(out=dec_sb, in_=dec_d)
        nc.vector.memset(ident1, 1.0)
        first_tiles = {}
        hT_t0 = hTp.tile([128, NCH, TT], F32R, tag="hT_t")
        qT_t0 = qTp.tile([128, NCH, TT], F32R, tag="qT_t")
        for j in range(NCH):
            (nc.sync if j % 2 == 0 else nc.gpsimd).dma_start(
                out=hT_t0[:, j, :], in_=hT_d[0, 0, j])
            (nc.scalar if j % 2 == 0 else nc.gpsimd).dma_start(
                out=qT_t0[:, j, :], in_=qT_d[0, 0, j])
        first_tiles[0] = (hT_t0, qT_t0)
        cov0 = smp.tile([128, TK], F32R, tag="cov")
        nc.vector.memset(cov0[:].bitcast(F32), 0.0)
        nc.gpsimd.dma_start(out=cov0[0:1, :], in_=cov_d[0:1, 0:TK])
        for m in range(1, MCH):
            nc.sync.dma_start(out=WhT_v[:, :, m, :], in_=WhT_d[m].rearrange("j p k -> p j k"))
            nc.scalar.dma_start(out=WqT_v[:, :, m, :], in_=WqT_d[m].rearrange("j p k -> p j k"))

        for b in range(BPC):
            wma = smp.tile([1, TK], F32, tag="wma")
            if b == 0:
                cov_t = cov0
            else:
                cov_t = smp.tile([128, TK], F32R, tag="cov")
                nc.vector.memset(cov_t[:].bitcast(F32), 0.0)
                nc.gpsimd.dma_start(out=cov_t[0:1, :], in_=cov_d[0:1, b * TK:(b + 1) * TK])
            ct0 = ps_ct.tile([1, 512], F32, tag="ct")
            ct1 = ps_ct.tile([1, 512], F32, tag="ct")

            # deferred-emission helper: c_t partial for tile tt (runs one
            # tt later so the exp/mask/transpose chain is off PE's critical
            # path)
            def emit_ct_partial(b, tt, wma=wma, ct0=ct0, ct1=ct1):
                sl = slice(tt * TT, (tt + 1) * TT)
                mk = thp.tile([1, TT], F32, tag="mk")
                nc.gpsimd.dma_start(
                    out=mk, in_=mask_d[0:1, b * TK + tt * TT: b * TK + (tt + 1) * TT])
                # wm = exp(scores) * mask  (in place)
                nc.vector.tensor_tensor(
                    out=wma[0:1, sl], in0=wma[0:1, sl], in1=mk, op=ALU.mult)
                # transpose wm tile -> [128, 4] chunks
                tp_ps = ps_tp.tile([128, 4], F32, tag="tp")
                for i in range(4):
                    tg(nc.tensor.transpose(
                        tp_ps[:, i:i + 1],
                        wma[0:1, tt * TT + i * 128: tt * TT + (i + 1) * 128],
                        ident1), "transp")
                wmT = thp.tile([128, 4], F32R, tag="wmT")
                nc.scalar.activation(wmT, tp_ps, AF.Identity)
                # c_t partial accumulation
                for half in range(2):
                    h_t = hp.tile([128, 2, N], F32R, tag="h_t")
                    nc.gpsimd.dma_start(
                        out=h_t,
                        in_=h_d[b].rearrange("(u p) n -> p u n", p=128)[
                            :, tt * 4 + half * 2: tt * 4 + half * 2 + 2, :])
                    for i in range(2):
                        u = half * 2 + i
                        tg(nc.tensor.matmul(ct0, wmT[:, u:u + 1], h_t[:, i, 0:512],
                                            start=(tt == 0 and u == 0),
                                            stop=(tt == NTT - 1 and u == 3)), "ct0")
                        tg(nc.tensor.matmul(ct1, wmT[:, u:u + 1], h_t[:, i, 512:1024],
                                            start=(tt == 0 and u == 0),
                                            stop=(tt == NTT - 1 and u == 3)), "ct1")

            # pending scores matvec, emitted one m-group behind so the
            # tanh (ACT) never stalls the PE: (sc_ps, m, tt, th_t)
            pending = []

            def flush_pending(after=None, pending=pending, wma=wma):
                sc_ps, m, tt, th_t = pending.pop()
                bi = tg(nc.tensor.matmul(sc_ps, v_sb[:, m:m + 1], th_t[:],
                                         start=(m == 0), stop=(m == MCH - 1)), "scores")
                if after is not None:
                    add_dep_helper(bi.ins, after.ins, sync=False,
                                   reason="hold scores matvec behind next group")
                if m == MCH - 1:
                    nc.scalar.activation(
                        wma[0:1, tt * TT:(tt + 1) * TT], sc_ps, AF.Exp)

            sc_ps = None
            for tt in range(NTT):
                sl = slice(tt * TT, (tt + 1) * TT)
                if b == 0 and tt == 0:
                    hT_t, qT_t = first_tiles.pop(0)
                else:
                    hT_t = hTp.tile([128, NCH, TT], F32R, tag="hT_t")
                    nc.sync.dma_start(out=hT_t, in_=hT_d[b, tt].rearrange("j p t -> p j t"))
                    qT_t = qTp.tile([128, NCH, TT], F32R, tag="qT_t")
                    nc.scalar.dma_start(out=qT_t, in_=qT_d[b, tt].rearrange("j p t -> p j t"))

                sc_ps = ps_sc.tile([1, TT], F32, tag="sc")
                for m in range(MCH):
                    at_ps = ps_at.tile([128, TT], F32, tag="at")
                    msl = slice(m * 128, (m + 1) * 128)
                    tg(nc.tensor.matmul(at_ps, WhT[:, 0, msl], hT_t[:, 0, :],
                                        start=True, stop=False), "projh0_start")
                    tg(nc.tensor.matmul(at_ps, Wc_sb[:, msl], cov_t[:, sl],
                                        start=False, stop=False), "rank1cov")
                    mm8 = None
                    for j in range(1, NCH):
                        bi = tg(nc.tensor.matmul(at_ps, WhT[:, j, msl], hT_t[:, j, :],
                                                 start=False, stop=False), f"projh{j}")
                        if j == NCH - 1:
                            mm8 = bi
                    for j in range(NCH):
                        tg(nc.tensor.matmul(at_ps, WqT[:, j, msl], qT_t[:, j, :],
                                            start=False, stop=(j == NCH - 1)), f"projq{j}")
                    if pending:
                        flush_pending(after=mm8)
                    th_t = thp.tile([128, TT], F32R, tag="th")
                    nc.scalar.activation(
                        th_t, at_ps, AF.Tanh,
                        bias=dec_sb[:, b * MCH + m: b * MCH + m + 1])
                    pending.append((sc_ps, m, tt, th_t))
                    # slot the deferred c_t work of the previous token tile
                    # behind this tile's projections
                    if m == 2 and tt > 0:
                        emit_ct_partial(b, tt - 1)

            flush_pending()
            emit_ct_partial(b, NTT - 1)

            # ---- softmax tail for batch b ----
            tot = smp.tile([1, 1], F32, tag="tot")
            nc.vector.reduce_sum(out=tot, in_=wma, axis=mybir.AxisListType.X)
            rt = smp.tile([1, 1], F32, tag="rt")
            nc.vector.reciprocal(rt, tot)
            nc.vector.tensor_scalar_mul(wma, wma, rt[0:1, 0:1])
            nc.sync.dma_start(out=attn_o[b], in_=wma)
            nc.vector.tensor_add(wma, wma, cov_t[0:1, :].bitcast(F32))
            nc.sync.dma_start(out=cvo_o[b], in_=wma)
            cts = smp.tile([1, N], F32, tag="cts")
            nc.scalar.activation(cts[0:1, 0:512], ct0, AF.Copy, scale=rt[0:1, 0:1])
            nc.scalar.activation(cts[0:1, 512:1024], ct1, AF.Copy, scale=rt[0:1, 0:1])
            nc.sync.dma_start(out=ct_o[b], in_=cts)

    nc.compile()
    return nc


def kernel(s_t_hat, h, enc_padding_mask, coverage, q_h, W_h, W_q, W_c, W_d, b_d, v_w):
    import os
    import jax
    from concourse import bass_utils

    try:
        jax.config.update("jax_compilation_cache_dir", "/tmp/jax_kernel_cache")
        jax.config.update("jax_persistent_cache_min_compile_time_secs", 0.0)
    except Exception:
        pass

    if "nc" not in _cache:
        _cache["nc"] = _build()
    nc = _cache["nc"]

    f32 = np.float32
    h = np.asarray(h, f32)
    q_h = np.asarray(q_h, f32)
    coverage = np.asarray(coverage, f32)
    enc_padding_mask = np.asarray(enc_padding_mask, f32)

    # host-side prep: tiled layouts [B, tt, j, p, t] for contiguous DMA
    hT = np.ascontiguousarray(
        h.reshape(B, NTT, TT, NCH, 128).transpose(0, 1, 3, 4, 2))
    qT = np.ascontiguousarray(
        q_h.reshape(B, NTT, TT, NCH, 128).transpose(0, 1, 3, 4, 2))
    dec = (np.asarray(s_t_hat, f32) @ np.asarray(W_d, f32).T
           + np.asarray(b_d, f32))                           # [B, N]
    WhT = np.ascontiguousarray(
        np.asarray(W_h, f32).reshape(MCH, 128, NCH, 128).transpose(0, 2, 3, 1))
    WqT = np.ascontiguousarray(
        np.asarray(W_q, f32).reshape(MCH, 128, NCH, 128).transpose(0, 2, 3, 1))
    v_arr = np.ascontiguousarray(np.asarray(v_w, f32).reshape(NCH, 128).T)
    Wc_arr = np.ascontiguousarray(np.asarray(W_c, f32).reshape(1, N))

    in_maps = []
    for c in range(NCORES):
        bs = slice(c * BPC, (c + 1) * BPC)
        dec_c = np.ascontiguousarray(
            dec[bs].reshape(BPC, MCH, 128).transpose(2, 0, 1).reshape(128, BPC * MCH))
        in_maps.append(dict(
            hT=np.ascontiguousarray(hT[bs]),
            qT=np.ascontiguousarray(qT[bs]),
            h=np.ascontiguousarray(h[bs]),
            WhT=WhT, WqT=WqT,
            cov=np.ascontiguousarray(coverage[bs].reshape(1, BPC * TK)),
            mask=np.ascontiguousarray(enc_padding_mask[bs].reshape(1, BPC * TK)),
            dec=dec_c, v=v_arr, Wc=Wc_arr,
        ))

    _cache["in_maps"] = in_maps
    res = bass_utils.run_bass_kernel_spmd(
        nc, in_maps, core_ids=list(range(NCORES)),
        trace=bool(os.environ.get("KERNEL_TRACE")))
    _cache["last_result"] = res

    c_t = np.concatenate([r["ct"] for r in res.results], axis=0)
    attn = np.concatenate([r["attn"] for r in res.results], axis=0)
    cvo = np.concatenate([r["cvo"] for r in res.results], axis=0)
    return (c_t, attn, cvo)



# revision 20
# speedup vs baseline: 1.1319x; 1.0320x over previous
"""Trainium2 Bass kernel for pointer-generator attention.

Problem (B=32, TK=2048, N=1024):
    enc_feat = h @ W_h.T                [B,TK,N]
    q_feat   = q_h @ W_q.T              [B,TK,N]
    dec_feat = s_t_hat @ W_d.T + b_d    [B,N]
    att  = tanh(enc_feat + dec_feat[:,None,:] + q_feat + coverage[...,None]*W_c)
    scores = att @ v_w                  [B,TK]
    attn = softmax(scores)*mask renormalized
    c_t  = attn @ h                     [B,N]
    coverage_out = coverage + attn

Strategy: data-parallel over batch, 4 batches per core on 8 cores.
Device kernel per batch (all matmuls in fp32r = bf16-rate with ~fp32
accuracy; weights stationary, host-pretransposed hT/qT as moving ops):
  - per 512-token tile: z^T[m,t] accumulated in PSUM over 16 K=128
    chunks (h and q) + a zero-padded K=128 rank-1 matmul for
    coverage*W_c (K=1 stationaries break LDWEIGHTS double-buffering);
    dec_feat folded in as the per-partition bias of the tanh ACT op;
    scores via PE matvec with v stationary;
    wm = exp(scores)*mask; c_t partial-accumulated with UNNORMALIZED wm
    (renormalization folded into the final copy as a 1/sum scale).
"""
import numpy as np

B, TK, N = 32, 2048, 1024
NCORES = 8
BPC = B // NCORES       # batches per core
TT = 512                # token tile
NTT = TK // TT          # 4 token tiles per batch
NCH = N // 128          # 8 contraction chunks
MCH = N // 128          # 8 output chunks

_cache = {}


def _build():
    from contextlib import ExitStack
    import concourse.bass as bassm
    import concourse.tile as tile
    from concourse import bacc, mybir
    from concourse.tile_rust import add_dep_helper

    F32 = mybir.dt.float32
    F32R = mybir.dt.float32r
    AF = mybir.ActivationFunctionType
    ALU = mybir.AluOpType

    nc = bacc.Bacc("TRN2", target_bir_lowering=False, debug=False,
                   enable_asserts=False, num_devices=NCORES)
    mmtags = _cache.setdefault("mmtags", {})

    def tg(bi, s):
        mmtags[bi.ins.name] = s
        return bi

    # ---- DRAM I/O ----
    hT_d = nc.dram_tensor("hT", [BPC, NTT, NCH, 128, TT], F32R, kind="ExternalInput").ap()
    qT_d = nc.dram_tensor("qT", [BPC, NTT, NCH, 128, TT], F32R, kind="ExternalInput").ap()
    h_d = nc.dram_tensor("h", [BPC, TK, N], F32R, kind="ExternalInput").ap()
    WhT_d = nc.dram_tensor("WhT", [MCH, NCH, 128, 128], F32R, kind="ExternalInput").ap()
    WqT_d = nc.dram_tensor("WqT", [MCH, NCH, 128, 128], F32R, kind="ExternalInput").ap()
    cov_d = nc.dram_tensor("cov", [1, BPC * TK], F32R, kind="ExternalInput").ap()
    mask_d = nc.dram_tensor("mask", [1, BPC * TK], F32, kind="ExternalInput").ap()
    dec_d = nc.dram_tensor("dec", [128, BPC * MCH], F32, kind="ExternalInput").ap()
    v_d = nc.dram_tensor("v", [128, NCH], F32R, kind="ExternalInput").ap()
    Wc_d = nc.dram_tensor("Wc", [128, NCH], F32, kind="ExternalInput").ap()

    ct_o = nc.dram_tensor("ct", [BPC, N], F32, kind="ExternalOutput").ap()
    attn_o = nc.dram_tensor("attn", [BPC, TK], F32, kind="ExternalOutput").ap()
    cvo_o = nc.dram_tensor("cvo", [BPC, TK], F32, kind="ExternalOutput").ap()

    with tile.TileContext(nc) as tc, ExitStack() as ctx:
        wp = ctx.enter_context(tc.tile_pool(name="wp", bufs=1))
        hTp = ctx.enter_context(tc.tile_pool(name="hTp", bufs=2))
        qTp = ctx.enter_context(tc.tile_pool(name="qTp", bufs=2))
        hp = ctx.enter_context(tc.tile_pool(name="hp", bufs=2))
        thp = ctx.enter_context(tc.tile_pool(name="thp", bufs=3))
        smp = ctx.enter_context(tc.tile_pool(name="smp", bufs=2))
        cvp = ctx.enter_context(tc.tile_pool(name="cvp", bufs=2))
        ps_at = ctx.enter_context(tc.tile_pool(name="ps_at", bufs=2, space="PSUM"))
        ps_sc = ctx.enter_context(tc.tile_pool(name="ps_sc", bufs=2, space="PSUM"))
        ps_tp = ctx.enter_context(tc.tile_pool(name="ps_tp", bufs=2, space="PSUM"))
        ps_ct = ctx.enter_context(tc.tile_pool(name="ps_ct", bufs=2, space="PSUM"))

        # ---- persistent weights / small tensors ----
        WhT = wp.tile([128, NCH, N], F32R)
        WqT = wp.tile([128, NCH, N], F32R)
        v_sb = wp.tile([128, NCH], F32R)
        Wc_sb = wp.tile([128, NCH], F32)
        dec_sb = wp.tile([128, BPC * MCH], F32)
        ident1 = wp.tile([1, 1], F32)
        # startup ordering: W arrives in m-slices so group m can start as
        # soon as slice m + the first token tiles are in (~14us), instead
        # of waiting for the full 8MB of weights
        WhT_v = WhT.rearrange("p j (m k) -> p j m k", k=128)
        WqT_v = WqT.rearrange("p j (m k) -> p j m k", k=128)
        nc.sync.dma_start(out=WhT_v[:, :, 0, :], in_=WhT_d[0].rearrange("j p k -> p j k"))
        nc.scalar.dma_start(out=WqT_v[:, :, 0, :], in_=WqT_d[0].rearrange("j p k -> p j k"))
        nc.gpsimd.dma_start(out=v_sb, in_=v_d)
        nc.gpsimd.dma_start(out=Wc_sb, in_=Wc_d)
        nc.gpsimd.dma_start(out=dec_sb, in_=dec_d)
        nc.vector.memset(ident1, 1.0)
        first_tiles = {}
        hT_t0 = hTp.tile([128, NCH, TT], F32R, tag="hT_t")
        qT_t0 = qTp.tile([128, NCH, TT], F32R, tag="qT_t")
        for j in range(NCH):
            (nc.sync if j % 2 == 0 else nc.gpsimd).dma_start(
                out=hT_t0[:, j, :], in_=hT_d[0, 0, j])
            (nc.scalar if j % 2 == 0 else nc.gpsimd).dma_start(
                out=qT_t0[:, j, :], in_=qT_d[0, 0, j])
        first_tiles[0] = (hT_t0, qT_t0)
        cov0 = smp.tile([1, TK], F32R, tag="cov")
        nc.gpsimd.dma_start(out=cov0, in_=cov_d[0:1, 0:TK])
        for m in range(1, MCH):
            nc.sync.dma_start(out=WhT_v[:, :, m, :], in_=WhT_d[m].rearrange("j p k -> p j k"))
            nc.scalar.dma_start(out=WqT_v[:, :, m, :], in_=WqT_d[m].rearrange("j p k -> p j k"))

        for b in range(BPC):
            wma = smp.tile([1, TK], F32, tag="wma")
            if b == 0:
                cov_t = cov0
            else:
                cov_t = smp.tile([1, TK], F32R, tag="cov")
                nc.gpsimd.dma_start(out=cov_t, in_=cov_d[0:1, b * TK:(b + 1) * TK])
            ct0 = ps_ct.tile([1, 512], F32, tag="ct")
            ct1 = ps_ct.tile([1, 512], F32, tag="ct")

            # deferred-emission helper: c_t partial for tile tt (runs one
            # tt later so the exp/mask/transpose chain is off PE's critical
            # path)
            def emit_ct_partial(b, tt, wma=wma, ct0=ct0, ct1=ct1):
                sl = slice(tt * TT, (tt + 1) * TT)
                mk = thp.tile([1, TT], F32, tag="mk")
                nc.gpsimd.dma_start(
                    out=mk, in_=mask_d[0:1, b * TK + tt * TT: b * TK + (tt + 1) * TT])
                # wm = exp(scores) * mask  (in place)
                nc.vector.tensor_tensor(
                    out=wma[0:1, sl], in0=wma[0:1, sl], in1=mk, op=ALU.mult)
                # transpose wm tile -> [128, 4] chunks
                tp_ps = ps_tp.tile([128, 4], F32, tag="tp")
                for i in range(4):
                    tg(nc.tensor.transpose(
                        tp_ps[:, i:i + 1],
                        wma[0:1, tt * TT + i * 128: tt * TT + (i + 1) * 128],
                        ident1), "transp")
                wmT = thp.tile([128, 4], F32R, tag="wmT")
                nc.scalar.activation(wmT, tp_ps, AF.Identity)
                # c_t partial accumulation
                for half in range(2):
                    h_t = hp.tile([128, 2, N], F32R, tag="h_t")
                    nc.gpsimd.dma_start(
                        out=h_t,
                        in_=h_d[b].rearrange("(u p) n -> p u n", p=128)[
                            :, tt * 4 + half * 2: tt * 4 + half * 2 + 2, :])
                    for i in range(2):
                        u = half * 2 + i
                        tg(nc.tensor.matmul(ct0, wmT[:, u:u + 1], h_t[:, i, 0:512],
                                            start=(tt == 0 and u == 0),
                                            stop=(tt == NTT - 1 and u == 3)), "ct0")
                        tg(nc.tensor.matmul(ct1, wmT[:, u:u + 1], h_t[:, i, 512:1024],
                                            start=(tt == 0 and u == 0),
                                            stop=(tt == NTT - 1 and u == 3)), "ct1")

            # pending scores matvec, emitted one m-group behind so the
            # tanh (ACT) never stalls the PE: (sc_ps, m, tt, th_t)
            pending = []

            def flush_pending(after=None, pending=pending, wma=wma):
                sc_ps, m, tt, th_t = pending.pop()
                bi = tg(nc.tensor.matmul(sc_ps, v_sb[:, m:m + 1], th_t[:],
                                         start=(m == 0), stop=(m == MCH - 1)), "scores")
                if after is not None:
                    add_dep_helper(bi.ins, after.ins, sync=False,
                                   reason="hold scores matvec behind next group")
                if m == MCH - 1:
                    nc.scalar.activation(
                        wma[0:1, tt * TT:(tt + 1) * TT], sc_ps, AF.Exp)

            sc_ps = None
            for tt in range(NTT):
                sl = slice(tt * TT, (tt + 1) * TT)
                if b == 0 and tt == 0:
                    hT_t, qT_t = first_tiles.pop(0)
                else:
                    hT_t = hTp.tile([128, NCH, TT], F32R, tag="hT_t")
                    nc.sync.dma_start(out=hT_t, in_=hT_d[b, tt].rearrange("j p t -> p j t"))
                    qT_t = qTp.tile([128, NCH, TT], F32R, tag="qT_t")
                    nc.scalar.dma_start(out=qT_t, in_=qT_d[b, tt].rearrange("j p t -> p j t"))

                cov_bc = cvp.tile([128, TT], F32, tag="cbc")
                _bsl = cov_d[0:1, b * TK + tt * TT: b * TK + (tt + 1) * TT]
                nc.gpsimd.dma_start(out=cov_bc, in_=bassm.AP(
                    tensor=_bsl.tensor, offset=_bsl.offset,
                    ap=[[0, 128]] + [list(x) for x in _bsl.ap[1:]]).bitcast(F32))
                sc_ps = ps_sc.tile([1, TT], F32, tag="sc")
                for m in range(MCH):
                    at_ps = ps_at.tile([128, TT], F32, tag="at")
                    msl = slice(m * 128, (m + 1) * 128)
                    tg(nc.tensor.matmul(at_ps, WhT[:, 0, msl], hT_t[:, 0, :],
                                        start=True, stop=False), "projh0_start")
                    mm8 = None
                    for j in range(1, NCH):
                        bi = tg(nc.tensor.matmul(at_ps, WhT[:, j, msl], hT_t[:, j, :],
                                                 start=False, stop=False), f"projh{j}")
                        if j == NCH - 1:
                            mm8 = bi
                    for j in range(NCH):
                        tg(nc.tensor.matmul(at_ps, WqT[:, j, msl], qT_t[:, j, :],
                                            start=False, stop=(j == NCH - 1)), f"projq{j}")
                    if pending:
                        flush_pending(after=mm8)
                    cw = cvp.tile([128, TT], F32, tag="cw")
                    nc.vector.tensor_scalar(
                        out=cw, in0=cov_bc, scalar1=Wc_sb[:, m:m + 1],
                        scalar2=None, op0=ALU.mult)
                    nc.vector.tensor_tensor(out=at_ps, in0=at_ps, in1=cw, op=ALU.add)
                    th_t = thp.tile([128, TT], F32R, tag="th")
                    nc.scalar.activation(
                        th_t, at_ps, AF.Tanh,
                        bias=dec_sb[:, b * MCH + m: b * MCH + m + 1])
                    pending.append((sc_ps, m, tt, th_t))
                    # slot the deferred c_t work of the previous token tile
                    # behind this tile's projections
                    if m == 2 and tt > 0:
                        emit_ct_partial(b, tt - 1)

            flush_pending()
            emit_ct_partial(b, NTT - 1)

            # ---- softmax tail for batch b ----
            tot = smp.tile([1, 1], F32, tag="tot")
            nc.vector.reduce_sum(out=tot, in_=wma, axis=mybir.AxisListType.X)
            rt = smp.tile([1, 1], F32, tag="rt")
            nc.vector.reciprocal(rt, tot)
            nc.vector.tensor_scalar_mul(wma, wma, rt[0:1, 0:1])
            nc.sync.dma_start(out=attn_o[b], in_=wma)
            nc.vector.tensor_add(wma, wma, cov_t[:].bitcast(F32))
            nc.sync.dma_start(out=cvo_o[b], in_=wma)
            cts = smp.tile([1, N], F32, tag="cts")
            nc.scalar.activation(cts[0:1, 0:512], ct0, AF.Copy, scale=rt[0:1, 0:1])
            nc.scalar.activation(cts[0:1, 512:1024], ct1, AF.Copy, scale=rt[0:1, 0:1])
            nc.sync.dma_start(out=ct_o[b], in_=cts)

    nc.compile()
    return nc


def kernel(s_t_hat, h, enc_padding_mask, coverage, q_h, W_h, W_q, W_c, W_d, b_d, v_w):
    import os
    import jax
    from concourse import bass_utils

    try:
        jax.config.update("jax_compilation_cache_dir", "/tmp/jax_kernel_cache")
        jax.config.update("jax_persistent_cache_min_compile_time_secs", 0.0)
    except Exception:
        pass

    if "nc" not in _cache:
        _cache["nc"] = _build()
    nc = _cache["nc"]

    f32 = np.float32
    h = np.asarray(h, f32)
    q_h = np.asarray(q_h, f32)
    coverage = np.asarray(coverage, f32)
    enc_padding_mask = np.asarray(enc_padding_mask, f32)

    # host-side prep: tiled layouts [B, tt, j, p, t] for contiguous DMA
    hT = np.ascontiguousarray(
        h.reshape(B, NTT, TT, NCH, 128).transpose(0, 1, 3, 4, 2))
    qT = np.ascontiguousarray(
        q_h.reshape(B, NTT, TT, NCH, 128).transpose(0, 1, 3, 4, 2))
    dec = (np.asarray(s_t_hat, f32) @ np.asarray(W_d, f32).T
           + np.asarray(b_d, f32))                           # [B, N]
    WhT = np.ascontiguousarray(
        np.asarray(W_h, f32).reshape(MCH, 128, NCH, 128).transpose(0, 2, 3, 1))
    WqT = np.ascontiguousarray(
        np.asarray(W_q, f32).reshape(MCH, 128, NCH, 128).transpose(0, 2, 3, 1))
    v_arr = np.ascontiguousarray(np.asarray(v_w, f32).reshape(NCH, 128).T)
    Wc_arr = np.ascontiguousarray(np.asarray(W_c, f32).reshape(NCH, 128).T)

    in_maps = []
    for c in range(NCORES):
        bs = slice(c * BPC, (c + 1) * BPC)
        dec_c = np.ascontiguousarray(
            dec[bs].reshape(BPC, MCH, 128).transpose(2, 0, 1).reshape(128, BPC * MCH))
        in_maps.append(dict(
            hT=np.ascontiguousarray(hT[bs]),
            qT=np.ascontiguousarray(qT[bs]),
            h=np.ascontiguousarray(h[bs]),
            WhT=WhT, WqT=WqT,
            cov=np.ascontiguousarray(coverage[bs].reshape(1, BPC * TK)),
            mask=np.ascontiguousarray(enc_padding_mask[bs].reshape(1, BPC * TK)),
            dec=dec_c, v=v_arr, Wc=Wc_arr,
        ))

    _cache["in_maps"] = in_maps
    res = bass_utils.run_bass_kernel_spmd(
        nc, in_maps, core_ids=list(range(NCORES)),
        trace=bool(os.environ.get("KERNEL_TRACE")))
    _cache["last_result"] = res

    c_t = np.concatenate([r["ct"] for r in res.results], axis=0)
    attn = np.concatenate([r["attn"] for r in res.results], axis=0)
    cvo = np.concatenate([r["cvo"] for r in res.results], axis=0)
    return (c_t, attn, cvo)

